# revision 46
# baseline (speedup 1.0000x reference)
"""MoE SwiGLU FFN (8 experts, top-2) + residual + LayerNorm on 8 Trainium2 cores.

Strategy: token-parallel with host-side routing/dispatch. The host computes the
router, assigns each token to one of 8 cores (1024 tokens per core, balanced so
every (core, expert) subset stays near-uniform), and builds per-core gathered
token matrices per expert. Each core computes all 8 experts over its own token
subsets -- expert matmuls in fp8 (e4m3) with DoubleRow perf mode (2 K-tiles per
pass), fp32 accumulate, dequant scales folded into the PSUM-evacuation ops.
The top-2 combine weight is folded into the Wo PSUM evacuation; the bias bo and
the residual are folded into a host-prepared xres tensor, so the device combine
is two indirect row gathers + adds + LayerNorm. Phase-2 work is emitted
interleaved into the expert loop (engine queues are FIFO) so the combine for
early token blocks overlaps the tail experts' matmuls.
"""

import math
import sys

import numpy as np

for p in ("/opt/trn_rl_repo",):
    if p not in sys.path:
        sys.path.insert(0, p)

import ml_dtypes

import concourse.bass as bass
import concourse.tile as tile
from concourse import bacc, mybir
from concourse.bass_utils import run_bass_kernel_spmd

EMBED = 512
HIDDEN_RAW = 1365  # floor(2*2048/3)
HIDDEN_PAD = 1408  # padded to 11*128 (zero-padded weights/biases)
NUM_EXPERTS = 8
NCORE = 8
TOP_K = 2
LN_EPS = 1e-5
TOK_PER_CORE = 1024
TOK_BLOCKS = TOK_PER_CORE // 128

F32 = mybir.dt.float32
BF16 = mybir.dt.bfloat16
I32 = mybir.dt.int32
F8 = mybir.dt.float8e4
NP_F8 = mybir.dt.np(F8)
DR = mybir.MatmulPerfMode.DoubleRow

SX = 8.0  # x quant scale
SW = 256.0  # weight quant scale
DEQ = 1.0 / (SX * SW)  # dequant after one fp8 matmul

_NC_CACHE: dict = {}


def _route_and_assign(flat: np.ndarray, router_w: np.ndarray):
    """Top-2 routing + balanced token->core assignment.

    Returns (e1, e2, w1, w2, assign, maxcnt): assign[t] in [0, 8) with exactly
    TOK_PER_CORE tokens per core and near-uniform per-(core, expert) subset
    sizes (maxcnt = the largest subset).
    """
    logits = flat.astype(np.float32) @ router_w.astype(np.float32)
    order = np.argsort(-logits, axis=-1, kind="stable")  # ties -> lower index
    e1 = order[:, 0].astype(np.int64)
    e2 = order[:, 1].astype(np.int64)
    v1 = np.take_along_axis(logits, order[:, :1], -1)[:, 0]
    v2 = np.take_along_axis(logits, order[:, :2], -1)[:, 1]
    m = np.maximum(v1, v2)
    a1 = np.exp(v1 - m)
    a2 = np.exp(v2 - m)
    s = a1 + a2
    w1 = (a1 / s).astype(np.float32)
    w2 = (a2 / s).astype(np.float32)

    n = flat.shape[0]
    cap = n // NCORE
    pair_key = e1 * NUM_EXPERTS + e2
    bucket_order = np.argsort(pair_key, kind="stable")
    core_total = np.zeros(NCORE, np.int64)
    core_expert = np.zeros((NCORE, NUM_EXPERTS), np.int64)
    assign = np.full(n, -1, np.int64)
    c = 0
    for t in bucket_order:
        best, bestcost = -1, None
        for step in range(NCORE):
            cc = (c + step) % NCORE
            if core_total[cc] >= cap:
                continue
            cost = (max(core_expert[cc, e1[t]], core_expert[cc, e2[t]]), core_total[cc])
            if bestcost is None or cost < bestcost:
                best, bestcost = cc, cost
        assign[t] = best
        core_total[best] += 1
        core_expert[best, e1[t]] += 1
        core_expert[best, e2[t]] += 1
        c = (best + 1) % NCORE
    return e1, e2, w1, w2, assign, int(core_expert.max())


def _build_nc(C: int, bounds: tuple, prefs: tuple, ln_affine: bool) -> bass.Bass:
    """Build the SPMD Bass program for per-(core,expert) capacity C.

    bounds[b] = highest expert index any of block b's tokens touches (max over
    cores); block b's combine is emitted right after that expert's compute.
    prefs[b] = number of 128-row blocks of expert bounds[b]'s (slot-sorted)
    rows that block b's gather can touch, so late gathers only depend on a
    prefix of the last expert's output blocks.
    """
    key = (C, bounds, prefs, ln_affine)
    if key in _NC_CACHE:
        return _NC_CACHE[key]
    BLK = (C + 127) // 128
    YROWS = NUM_EXPERTS * C

    nc = bacc.Bacc(None, target_bir_lowering=False)
    xt = nc.declare_dram_parameter("xt", [NUM_EXPERTS, 128, 4, C], F8, isOutput=False)
    wv = nc.declare_dram_parameter("wv", [NUM_EXPERTS, 128, 4, HIDDEN_PAD], F8, isOutput=False)
    wg = nc.declare_dram_parameter("wg", [NUM_EXPERTS, 128, 4, HIDDEN_PAD], F8, isOutput=False)
    wo = nc.declare_dram_parameter("wo", [NUM_EXPERTS, 128, 11, EMBED], F8, isOutput=False)
    bvt = nc.declare_dram_parameter("bvt", [NUM_EXPERTS, 128, 11], F32, isOutput=False)
    bgt8 = nc.declare_dram_parameter("bgt8", [NUM_EXPERTS, 128, 11], F32, isOutput=False)
    wesc = nc.declare_dram_parameter("wesc", [128, NUM_EXPERTS * BLK], F32, isOutput=False)
    gam = nc.declare_dram_parameter("gamma", [128, EMBED], F32, isOutput=False)
    bet = nc.declare_dram_parameter("beta", [128, EMBED], F32, isOutput=False)
    xres = nc.declare_dram_parameter("xres", [TOK_BLOCKS, 128, EMBED], F32, isOutput=False)
    idx = nc.declare_dram_parameter("idx", [128, TOK_BLOCKS, 2], I32, isOutput=False)
    out = nc.declare_dram_parameter("out", [TOK_BLOCKS, 128, EMBED], F32, isOutput=True)
    # row for (expert e, block blk, partition p) = e*BLK*128 + p*BLK + blk, so
    # each expert's 3 blocks write with ONE DMA from a [128, BLK, EMBED] tile
    ydram = nc.dram_tensor("ydram", [NUM_EXPERTS * BLK * 128, EMBED], BF16)

    from contextlib import ExitStack

    with tile.TileContext(nc) as tc, ExitStack() as ctx:
        const = ctx.enter_context(tc.tile_pool(name="const", bufs=1))
        wpool = ctx.enter_context(tc.tile_pool(name="w", bufs=3))
        xpool = ctx.enter_context(tc.tile_pool(name="x", bufs=3))
        hpool = ctx.enter_context(tc.tile_pool(name="h", bufs=2))
        vgpool = ctx.enter_context(tc.tile_pool(name="vg", bufs=3))
        ypool = ctx.enter_context(tc.tile_pool(name="y", bufs=3))
        pvg = ctx.enter_context(tc.tile_pool(name="pvg", bufs=3, space="PSUM"))
        pop = ctx.enter_context(tc.tile_pool(name="pop", bufs=2, space="PSUM"))
        c2pool = ctx.enter_context(tc.tile_pool(name="c2", bufs=2))
        s01pool = ctx.enter_context(tc.tile_pool(name="s01", bufs=TOK_BLOCKS))
        mvpool = ctx.enter_context(tc.tile_pool(name="mv", bufs=TOK_BLOCKS))

        # constants: allocated up front; DMAs emitted after the first expert's
        # weight loads start so they don't steal startup HBM bandwidth.
        gam_t = const.tile([128, EMBED], F32)
        bet_t = const.tile([128, EMBED], F32)
        eps_t = const.tile([128, 1], F32)
        nc.vector.memset(eps_t, LN_EPS)
        idx_t = const.tile([128, TOK_BLOCKS, 2], I32)
        wesc_t = const.tile([128, NUM_EXPERTS * BLK], F32)
        var_all = const.tile([128, TOK_BLOCKS], F32)  # per-block variance
        rs_all = const.tile([128, TOK_BLOCKS], F32)
        warm_t = const.tile([128, 1], F32)
        nc.scalar.activation(out=warm_t, in_=eps_t, func=mybir.ActivationFunctionType.Silu, bias=0.0, scale=1.0)
        nc.scalar.activation(out=warm_t, in_=eps_t, func=mybir.ActivationFunctionType.Identity, bias=0.0, scale=1.0)

        s01_tiles: dict = {}

        def emit_gather_adds(b):
            """Gather the token block's two expert rows and sum with the
            residual. Emitted inside expert bounds[b]'s Wo loop, right after
            the ydram block write the gather actually depends on (prefs[b]),
            so the chain completes by the end of the expert's section."""
            xr_t = c2pool.tile([128, EMBED], F32, tag="xr")
            nc.sync.dma_start(out=xr_t, in_=xres[b])
            yrows = bounds[b] * BLK * 128 + prefs[b] * 128
            y1 = c2pool.tile([128, EMBED], BF16, tag="y1")
            nc.gpsimd.indirect_dma_start(
                out=y1,
                out_offset=None,
                in_=ydram[:yrows],
                in_offset=bass.IndirectOffsetOnAxis(ap=idx_t[:, b, 0:1], axis=0),
            )
            y2 = c2pool.tile([128, EMBED], BF16, tag="y2")
            nc.gpsimd.indirect_dma_start(
                out=y2,
                out_offset=None,
                in_=ydram[:yrows],
                in_offset=bass.IndirectOffsetOnAxis(ap=idx_t[:, b, 1:2], axis=0),
            )
            s01 = s01pool.tile([128, EMBED], F32, tag="s01")
            # tail blocks (bound = last expert) sum on vector so the adds run
            # in parallel with the gpsimd gathers after phase 1 ends
            eng_add = nc.vector if bounds[b] == NUM_EXPERTS - 1 else nc.gpsimd
            eng_add.tensor_tensor(out=s01, in0=y1, in1=y2, op=mybir.AluOpType.add)
            eng_add.tensor_tensor(out=s01, in0=s01, in1=xr_t, op=mybir.AluOpType.add)
            s01_tiles[b] = s01

        def emit_stats(b):
            """LN stats on vector, emitted at the end of expert bounds[b]'s
            section; by then the gather+add chain is already done, so this
            never head-of-line blocks the next expert's h-muls."""
            s01 = s01_tiles[b]
            stats = c2pool.tile([128, 6], F32, tag="st")
            nc.vector.bn_stats(out=stats, in_=s01)
            mv_b = mvpool.tile([128, 2], F32, tag="mv")
            nc.vector.bn_aggr(out=mv_b, in_=stats)
            nc.vector.tensor_scalar_mul(out=var_all[:, b : b + 1], in0=mv_b[:, 1:2], scalar1=1.0)
            s01_tiles[b] = (s01, mv_b)

        # Phase 1: per-expert SwiGLU MLP over the gathered token subsets.
        for e in range(NUM_EXPERTS):
            xt_t = xpool.tile([128, 4, C], F8, tag="xt")
            bvt_t = wpool.tile([128, 11], F32, tag="bvt")
            bgt8_t = wpool.tile([128, 11], F32, tag="bgt8")
            wv_t = wpool.tile([128, 4, HIDDEN_PAD], F8, tag="wv")
            wg_t = wpool.tile([128, 4, HIDDEN_PAD], F8, tag="wg")
            wo_t = wpool.tile([128, 11, EMBED], F8, tag="wo")
            if e == 0:
                # split the startup-critical loads so the first K-tiles'
                # matmuls can begin while the second halves stream in
                nc.sync.dma_start(out=xt_t[:, :2], in_=xt[e, :, :2])
                nc.sync.dma_start(out=wv_t[:, :2], in_=wv[e, :, :2])
                nc.sync.dma_start(out=xt_t[:, 2:], in_=xt[e, :, 2:])
                nc.sync.dma_start(out=wv_t[:, 2:], in_=wv[e, :, 2:])
                nc.sync.dma_start(out=bvt_t, in_=bvt[e])
                nc.sync.dma_start(out=bgt8_t, in_=bgt8[e])
                nc.sync.dma_start(out=wg_t, in_=wg[e])
                nc.sync.dma_start(out=wo_t, in_=wo[e])
            else:
                nc.sync.dma_start(out=xt_t, in_=xt[e])
                nc.sync.dma_start(out=bvt_t, in_=bvt[e])
                nc.sync.dma_start(out=bgt8_t, in_=bgt8[e])
                nc.sync.dma_start(out=wv_t, in_=wv[e])
                nc.sync.dma_start(out=wg_t, in_=wg[e])
                nc.sync.dma_start(out=wo_t, in_=wo[e])
            if e == 0:
                nc.scalar.dma_start(out=idx_t, in_=idx[:, :, :])
                nc.scalar.dma_start(out=wesc_t, in_=wesc[:, :])
                if ln_affine:
                    nc.scalar.dma_start(out=gam_t, in_=gam[:, :])
                    nc.scalar.dma_start(out=bet_t, in_=bet[:, :])

            # one h tile per Wo k-pair so each Wo matmul depends only on its
            # own pair's muls (a single h tile makes Wo j=0 falsely wait for
            # the m=10 elementwise chain, idling the PE ~6us per expert)
            h_pairs = [
                hpool.tile([128, 2, C], F8, tag=f"h{j}", name=f"h_pair{j}_{e}")
                for j in range(5)
            ]
            h10 = hpool.tile([128, C], F8, tag="h10")
            for m in range(11):
                psv = pvg.tile([128, C], F32, tag="psv")
                psg = pvg.tile([128, C], F32, tag="psg")
                for i in range(2):
                    nc.tensor.matmul(
                        psv,
                        lhsT=wv_t[:, 2 * i : 2 * i + 2, m * 128 : (m + 1) * 128],
                        rhs=xt_t[:, 2 * i : 2 * i + 2, :],
                        start=(i == 0),
                        stop=(i == 1),
                        perf_mode=DR,
                    )
                for i in range(2):
                    nc.tensor.matmul(
                        psg,
                        lhsT=wg_t[:, 2 * i : 2 * i + 2, m * 128 : (m + 1) * 128],
                        rhs=xt_t[:, 2 * i : 2 * i + 2, :],
                        start=(i == 0),
                        stop=(i == 1),
                        perf_mode=DR,
                    )
                v_t = vgpool.tile([128, C], BF16, tag="v")
                nc.scalar.activation(
                    out=v_t,
                    in_=psv,
                    func=mybir.ActivationFunctionType.Silu,
                    bias=bvt_t[:, m : m + 1],
                    scale=DEQ,
                )
                g_t = vgpool.tile([128, C], BF16, tag="g")
                if m % 2 == 1:
                    # scalar engine: Identity(psg/256 + 8*bg) == the g dequant
                    nc.scalar.activation(
                        out=g_t,
                        in_=psg,
                        func=mybir.ActivationFunctionType.Identity,
                        bias=bgt8_t[:, m : m + 1],
                        scale=SX * DEQ,
                    )
                else:
                    nc.vector.tensor_scalar(
                        out=g_t,
                        in0=psg,
                        scalar1=SX * DEQ,  # 1/256: folds the x8 used for h's fp8 range
                        scalar2=bgt8_t[:, m : m + 1],
                        op0=mybir.AluOpType.mult,
                        op1=mybir.AluOpType.add,
                    )
                h_dst = h10 if m == 10 else h_pairs[m // 2][:, m % 2, :]
                nc.vector.tensor_tensor(
                    out=h_dst, in0=v_t, in1=g_t, op=mybir.AluOpType.mult
                )

            y_all = ypool.tile([128, BLK, EMBED], BF16, tag="y")
            if C < BLK * 128:  # ragged last block: zero its column first
                nc.vector.memset(y_all[:, BLK - 1, :], 0.0)
            for blk in range(BLK):
                mb = min(128, C - blk * 128)  # last block is ragged
                pso = pop.tile([128, EMBED], F32, tag="pso")
                for j in range(5):
                    nc.tensor.matmul(
                        pso[:mb],
                        lhsT=h_pairs[j][:, :, blk * 128 : blk * 128 + mb],
                        rhs=wo_t[:, 2 * j : 2 * j + 2, :],
                        start=(j == 0),
                        stop=False,
                        perf_mode=DR,
                    )
                nc.tensor.matmul(
                    pso[:mb],
                    lhsT=h10[:, blk * 128 : blk * 128 + mb],
                    rhs=wo_t[:, 10, :],
                    start=False,
                    stop=True,
                )
                # PSUM evac with dequant*combine-weight scale; engine varies
                # per block to balance load (single scalar-AP ops are fast on
                # any engine)
                col = e * BLK + blk
                if blk == 0:
                    nc.scalar.activation(
                        out=y_all[:mb, blk, :],
                        in_=pso[:mb],
                        func=mybir.ActivationFunctionType.Identity,
                        bias=0.0,
                        scale=wesc_t[:mb, col : col + 1],
                    )
                else:
                    # gpsimd cannot read PSUM; vector takes blocks 1 and 2
                    nc.vector.tensor_scalar_mul(
                        out=y_all[:mb, blk, :],
                        in0=pso[:mb],
                        scalar1=wesc_t[:mb, col : col + 1],
                    )
                r0 = e * BLK * 128 + blk * 128
                nc.gpsimd.dma_start(out=ydram[r0 : r0 + 128, :], in_=y_all[:, blk, :])
                for b in range(TOK_BLOCKS):
                    if bounds[b] == e and prefs[b] == blk + 1:
                        emit_gather_adds(b)

            for b in range(TOK_BLOCKS):
                if bounds[b] == e:
                    emit_stats(b)

        # Phase 2b: batched rsqrt (single Sqrt table load after all silus),
        # then per-block normalize + output, alternating vector/gpsimd.
        nc.scalar.activation(
            out=rs_all,
            in_=var_all,
            func=mybir.ActivationFunctionType.Sqrt,
            bias=eps_t,
            scale=1.0,
        )
        nc.vector.reciprocal(out=rs_all, in_=rs_all)
        for b in range(TOK_BLOCKS):
            s01, mv_b = s01_tiles[b]
            # offset-0 [128,1] scalar APs only -- offset/strided scalar APs
            # trigger a ~4-8x DVE slow path on hardware
            rs_b = c2pool.tile([128, 1], F32, tag="rsb")
            nc.vector.tensor_scalar_mul(out=rs_b, in0=rs_all[:, b : b + 1], scalar1=1.0)
            nrm = c2pool.tile([128, EMBED], F32, tag=f"nrm{b % 2}")
            nc.vector.tensor_scalar(
                out=nrm,
                in0=s01,
                scalar1=mv_b[:, 0:1],
                scalar2=rs_b,
                op0=mybir.AluOpType.subtract,
                op1=mybir.AluOpType.mult,
            )
            if ln_affine:
                nc.vector.tensor_mul(out=nrm, in0=nrm, in1=gam_t)
                nc.vector.tensor_add(out=nrm, in0=nrm, in1=bet_t)
            nc.sync.dma_start(out=out[b], in_=nrm)

    nc.finalize()
    _NC_CACHE[key] = nc
    return nc


def prepare(x, router_w, Wv, bv, Wg, bg, Wo, bo, gamma, beta):
    """Host-side routing, balancing, quantization, per-core input build."""
    x = np.asarray(x)
    router_w = np.asarray(router_w, dtype=np.float32)
    Wv = np.asarray(Wv, dtype=np.float32)
    bv = np.asarray(bv, dtype=np.float32)
    Wg = np.asarray(Wg, dtype=np.float32)
    bg = np.asarray(bg, dtype=np.float32)
    Wo = np.asarray(Wo, dtype=np.float32)
    bo = np.asarray(bo, dtype=np.float32)
    gamma = np.asarray(gamma, dtype=np.float32)
    beta = np.asarray(beta, dtype=np.float32)

    orig_shape = x.shape
    flat = x.reshape(-1, EMBED).astype(np.float32)
    n = flat.shape[0]
    assert n == NCORE * TOK_PER_CORE

    e1, e2, w1, w2, assign, maxcnt = _route_and_assign(flat, router_w)
    C = max(256, ((maxcnt + 15) // 16) * 16)
    BLK = (C + 127) // 128

    # process experts in descending-load order: the last-processed expert is
    # the lightest, so fewer token blocks are bound to it and the combine tail
    # after the final expert shrinks
    loads = np.bincount(np.concatenate([e1, e2]), minlength=NUM_EXPERTS)
    perm = np.argsort(-loads, kind="stable")  # slot s processes expert perm[s]
    pos = np.empty(NUM_EXPERTS, np.int64)
    pos[perm] = np.arange(NUM_EXPERTS)

    # replicated weights, pre-tiled to [e, 128, ktiles, free]; fp8 with scale SW
    wv_r = np.zeros((NUM_EXPERTS, EMBED, HIDDEN_PAD), np.float32)
    wv_r[:, :, :HIDDEN_RAW] = Wv * SW
    wg_r = np.zeros((NUM_EXPERTS, EMBED, HIDDEN_PAD), np.float32)
    wg_r[:, :, :HIDDEN_RAW] = Wg * SW
    wo_r = np.zeros((NUM_EXPERTS, HIDDEN_PAD, EMBED), np.float32)
    wo_r[:, :HIDDEN_RAW, :] = Wo * SW
    wv_tiled = np.ascontiguousarray(
        wv_r.reshape(NUM_EXPERTS, 4, 128, HIDDEN_PAD).transpose(0, 2, 1, 3).astype(NP_F8)
    )
    wg_tiled = np.ascontiguousarray(
        wg_r.reshape(NUM_EXPERTS, 4, 128, HIDDEN_PAD).transpose(0, 2, 1, 3).astype(NP_F8)
    )
    wo_tiled = np.ascontiguousarray(
        wo_r.reshape(NUM_EXPERTS, 11, 128, EMBED).transpose(0, 2, 1, 3).astype(NP_F8)
    )
    bv_pad = np.zeros((NUM_EXPERTS, HIDDEN_PAD), np.float32)
    bv_pad[:, :HIDDEN_RAW] = bv
    bg_pad = np.zeros((NUM_EXPERTS, HIDDEN_PAD), np.float32)
    bg_pad[:, :HIDDEN_RAW] = bg * SX  # g is kept scaled by SX so h=v*g lands at 8*h
    # [e, 128, 11]: column m holds the bias slice for H-tile m on partitions
    bvt = np.ascontiguousarray(bv_pad.reshape(NUM_EXPERTS, 11, 128).transpose(0, 2, 1))
    bgt8 = np.ascontiguousarray(bg_pad.reshape(NUM_EXPERTS, 11, 128).transpose(0, 2, 1))
    # reindex weights/biases by processing slot
    wv_tiled = np.ascontiguousarray(wv_tiled[perm])
    wg_tiled = np.ascontiguousarray(wg_tiled[perm])
    wo_tiled = np.ascontiguousarray(wo_tiled[perm])
    bvt = np.ascontiguousarray(bvt[perm])
    bgt8 = np.ascontiguousarray(bgt8[perm])
    gam_rep = np.ascontiguousarray(np.broadcast_to(gamma, (128, EMBED)))
    bet_rep = np.ascontiguousarray(np.broadcast_to(beta, (128, EMBED)))

    # bo folded into the residual: xres = x + w1*bo[e1] + w2*bo[e2]
    bo_comb = w1[:, None] * bo[e1] + w2[:, None] * bo[e2]
    res_full = flat + bo_comb

    in_maps = []
    core_token_ids = []
    per_core_sorted = []
    block_bound = np.zeros(TOK_BLOCKS, np.int64)
    for c in range(NCORE):
        tok_c = np.nonzero(assign == c)[0]
        assert tok_c.size == TOK_PER_CORE
        # order the core's tokens by the latest processing SLOT they touch, so
        # early blocks' combines only depend on a prefix of the expert loop
        emax = np.maximum(pos[e1[tok_c]], pos[e2[tok_c]])
        order = np.argsort(emax, kind="stable")
        tok_c = tok_c[order]
        per_core_sorted.append(tok_c)
        emax_sorted = emax[order]
        for b in range(TOK_BLOCKS):
            block_bound[b] = max(block_bound[b], emax_sorted[(b + 1) * 128 - 1])
    bounds = tuple(int(v) for v in block_bound)
    # prefs[b]: how many 128-row blocks of expert bounds[b]'s slot-sorted rows
    # block b's gather needs (max over cores, block-aligned)
    block_pref = np.zeros(TOK_BLOCKS, np.int64)
    for c in range(NCORE):
        tok_c = per_core_sorted[c]
        for b in range(TOK_BLOCKS):
            eb = perm[bounds[b]]
            sel = (e1[tok_c] == eb) | (e2[tok_c] == eb)
            ids = np.nonzero(sel)[0]
            pref = int(np.searchsorted(ids, (b + 1) * 128))
            block_pref[b] = max(block_pref[b], (pref + 127) // 128)
    prefs = tuple(int(v) for v in block_pref)

    x8 = (flat * SX).astype(NP_F8)
    for c in range(NCORE):
        tok_c = per_core_sorted[c]
        core_token_ids.append(tok_c)
        # per-expert gathered token subsets (fp8, pre-scaled)
        xt_c = np.zeros((NUM_EXPERTS, EMBED, C), NP_F8)
        ridx = np.zeros((TOK_PER_CORE, 2), np.int64)
        wesc_c = np.zeros((128, NUM_EXPERTS * BLK), np.float32)
        for s in range(NUM_EXPERTS):
            ex = perm[s]
            sel1 = e1[tok_c] == ex
            sel2 = e2[tok_c] == ex
            # slot-sorted rows: slot s's row r serves the r-th smallest token
            # slot touching it, so a token block's gather needs only a prefix
            ids = np.nonzero(sel1 | sel2)[0]
            cnt = ids.size
            assert cnt <= C, (cnt, C)
            xt_c[s, :, :cnt] = x8[tok_c[ids]].T
            rows = s * BLK * 128 + np.arange(cnt)
            first = sel1[ids]
            ridx[ids[first], 0] = rows[first]
            ridx[ids[~first], 1] = rows[~first]
            wvals = np.where(first, w1[tok_c[ids]], w2[tok_c[ids]])
            wecol = np.zeros(BLK * 128, np.float32)
            wecol[:cnt] = wvals * DEQ
            wesc_c[:, s * BLK : (s + 1) * BLK] = wecol.reshape(BLK, 128).T
        in_map = {
            "xt": np.ascontiguousarray(
                xt_c.reshape(NUM_EXPERTS, 4, 128, C).transpose(0, 2, 1, 3)
            ),
            "wv": wv_tiled,
            "wg": wg_tiled,
            "wo": wo_tiled,
            "bvt": bvt,
            "bgt8": bgt8,
            "wesc": wesc_c,
            "gamma": gam_rep,
            "beta": bet_rep,
            "xres": np.ascontiguousarray(
                res_full[tok_c].reshape(TOK_BLOCKS, 128, EMBED)
            ),
            # [128, TOK_BLOCKS, 2]: [p, b, s] = ydram row for token b*128+p slot s
            "idx": np.ascontiguousarray(
                ridx.reshape(TOK_BLOCKS, 128, 2).transpose(1, 0, 2).astype(np.int32)
            ),
        }
        in_maps.append(in_map)

    ln_affine = not (np.all(gamma == 1.0) and np.all(beta == 0.0))
    return in_maps, C, bounds, prefs, ln_affine, core_token_ids, orig_shape


def assemble(results, core_token_ids, orig_shape):
    """Scatter per-core outputs back to full token order."""
    n = NCORE * TOK_PER_CORE
    out_full = np.zeros((n, EMBED), np.float32)
    for c in range(NCORE):
        out_full[core_token_ids[c]] = results[c]["out"].reshape(TOK_PER_CORE, EMBED)
    return out_full.reshape(orig_shape)


def kernel(x, router_w, Wv, bv, Wg, bg, Wo, bo, gamma, beta):
    in_maps, C, bounds, prefs, ln_affine, core_token_ids, orig_shape = prepare(
        x, router_w, Wv, bv, Wg, bg, Wo, bo, gamma, beta
    )
    nc = _build_nc(C, bounds, prefs, ln_affine)
    res = run_bass_kernel_spmd(nc, in_maps, list(range(NCORE)))
    return assemble(res.results, core_token_ids, orig_shape)


# revision 49
# speedup vs baseline: 1.0368x; 1.0368x over previous
"""MoE SwiGLU FFN (8 experts, top-2) + residual + LayerNorm on 8 Trainium2 cores.

Strategy: token-parallel with host-side routing/dispatch. The host computes the
router, assigns each token to one of 8 cores (1024 tokens per core, balanced so
every (core, expert) subset stays near-uniform), and builds per-core gathered
token matrices per expert. Each core computes all 8 experts over its own token
subsets -- expert matmuls in fp8 (e4m3) with DoubleRow perf mode (2 K-tiles per
pass), fp32 accumulate, dequant scales folded into the PSUM-evacuation ops.
The top-2 combine weight is folded into the Wo PSUM evacuation; the bias bo and
the residual are folded into a host-prepared xres tensor, so the device combine
is two indirect row gathers + adds + LayerNorm. Phase-2 work is emitted
interleaved into the expert loop (engine queues are FIFO) so the combine for
early token blocks overlaps the tail experts' matmuls.
"""

import math
import sys

import numpy as np

for p in ("/opt/trn_rl_repo",):
    if p not in sys.path:
        sys.path.insert(0, p)

import ml_dtypes

import concourse.bass as bass
import concourse.tile as tile
from concourse import bacc, mybir
from concourse.bass_utils import run_bass_kernel_spmd

EMBED = 512
HIDDEN_RAW = 1365  # floor(2*2048/3)
HIDDEN_PAD = 1408  # padded to 11*128 (zero-padded weights/biases)
NUM_EXPERTS = 8
NCORE = 8
TOP_K = 2
LN_EPS = 1e-5
TOK_PER_CORE = 1024
TOK_BLOCKS = TOK_PER_CORE // 128

F32 = mybir.dt.float32
BF16 = mybir.dt.bfloat16
I32 = mybir.dt.int32
F8 = mybir.dt.float8e4
NP_F8 = mybir.dt.np(F8)
DR = mybir.MatmulPerfMode.DoubleRow

SX = 8.0  # x quant scale
SW = 256.0  # weight quant scale
DEQ = 1.0 / (SX * SW)  # dequant after one fp8 matmul

_NC_CACHE: dict = {}


def _route_and_assign(flat: np.ndarray, router_w: np.ndarray):
    """Top-2 routing + balanced token->core assignment.

    Returns (e1, e2, w1, w2, assign, maxcnt): assign[t] in [0, 8) with exactly
    TOK_PER_CORE tokens per core and near-uniform per-(core, expert) subset
    sizes (maxcnt = the largest subset).
    """
    logits = flat.astype(np.float32) @ router_w.astype(np.float32)
    order = np.argsort(-logits, axis=-1, kind="stable")  # ties -> lower index
    e1 = order[:, 0].astype(np.int64)
    e2 = order[:, 1].astype(np.int64)
    v1 = np.take_along_axis(logits, order[:, :1], -1)[:, 0]
    v2 = np.take_along_axis(logits, order[:, :2], -1)[:, 1]
    m = np.maximum(v1, v2)
    a1 = np.exp(v1 - m)
    a2 = np.exp(v2 - m)
    s = a1 + a2
    w1 = (a1 / s).astype(np.float32)
    w2 = (a2 / s).astype(np.float32)

    n = flat.shape[0]
    cap = n // NCORE
    pair_key = e1 * NUM_EXPERTS + e2
    bucket_order = np.argsort(pair_key, kind="stable")
    core_total = np.zeros(NCORE, np.int64)
    core_expert = np.zeros((NCORE, NUM_EXPERTS), np.int64)
    assign = np.full(n, -1, np.int64)
    c = 0
    for t in bucket_order:
        best, bestcost = -1, None
        for step in range(NCORE):
            cc = (c + step) % NCORE
            if core_total[cc] >= cap:
                continue
            cost = (max(core_expert[cc, e1[t]], core_expert[cc, e2[t]]), core_total[cc])
            if bestcost is None or cost < bestcost:
                best, bestcost = cc, cost
        assign[t] = best
        core_total[best] += 1
        core_expert[best, e1[t]] += 1
        core_expert[best, e2[t]] += 1
        c = (best + 1) % NCORE
    return e1, e2, w1, w2, assign, int(core_expert.max())


def _build_nc(C: int, bounds: tuple, prefs: tuple, ln_affine: bool) -> bass.Bass:
    """Build the SPMD Bass program for per-(core,expert) capacity C.

    bounds[b] = highest expert index any of block b's tokens touches (max over
    cores); block b's combine is emitted right after that expert's compute.
    prefs[b] = number of 128-row blocks of expert bounds[b]'s (slot-sorted)
    rows that block b's gather can touch, so late gathers only depend on a
    prefix of the last expert's output blocks.
    """
    key = (C, bounds, prefs, ln_affine)
    if key in _NC_CACHE:
        return _NC_CACHE[key]
    BLK = (C + 127) // 128
    YROWS = NUM_EXPERTS * C

    nc = bacc.Bacc(None, target_bir_lowering=False)
    xt = nc.declare_dram_parameter("xt", [NUM_EXPERTS, 128, 4, C], F8, isOutput=False)
    wv = nc.declare_dram_parameter("wv", [NUM_EXPERTS, 128, 4, HIDDEN_PAD], F8, isOutput=False)
    wg = nc.declare_dram_parameter("wg", [NUM_EXPERTS, 128, 4, HIDDEN_PAD], F8, isOutput=False)
    wo = nc.declare_dram_parameter("wo", [NUM_EXPERTS, 128, 11, EMBED], F8, isOutput=False)
    bvt = nc.declare_dram_parameter("bvt", [NUM_EXPERTS, 128, 11], F32, isOutput=False)
    bgt8 = nc.declare_dram_parameter("bgt8", [NUM_EXPERTS, 128, 11], F32, isOutput=False)
    wesc = nc.declare_dram_parameter("wesc", [128, NUM_EXPERTS * BLK], F32, isOutput=False)
    gam = nc.declare_dram_parameter("gamma", [128, EMBED], F32, isOutput=False)
    bet = nc.declare_dram_parameter("beta", [128, EMBED], F32, isOutput=False)
    xres = nc.declare_dram_parameter("xres", [TOK_BLOCKS, 128, EMBED], F32, isOutput=False)
    idx = nc.declare_dram_parameter("idx", [128, TOK_BLOCKS, 2], I32, isOutput=False)
    out = nc.declare_dram_parameter("out", [TOK_BLOCKS, 128, EMBED], F32, isOutput=True)
    # row for (expert e, block blk, partition p) = e*BLK*128 + p*BLK + blk, so
    # each expert's 3 blocks write with ONE DMA from a [128, BLK, EMBED] tile
    ydram = nc.dram_tensor("ydram", [NUM_EXPERTS * BLK * 128, EMBED], BF16)

    from contextlib import ExitStack

    with tile.TileContext(nc) as tc, ExitStack() as ctx:
        const = ctx.enter_context(tc.tile_pool(name="const", bufs=1))
        wpool = ctx.enter_context(tc.tile_pool(name="w", bufs=3))
        xpool = ctx.enter_context(tc.tile_pool(name="x", bufs=3))
        hpool = ctx.enter_context(tc.tile_pool(name="h", bufs=2))
        vgpool = ctx.enter_context(tc.tile_pool(name="vg", bufs=3))
        ypool = ctx.enter_context(tc.tile_pool(name="y", bufs=3))
        pvg = ctx.enter_context(tc.tile_pool(name="pvg", bufs=3, space="PSUM"))
        pop = ctx.enter_context(tc.tile_pool(name="pop", bufs=2, space="PSUM"))
        c2pool = ctx.enter_context(tc.tile_pool(name="c2", bufs=2))
        s01pool = ctx.enter_context(tc.tile_pool(name="s01", bufs=TOK_BLOCKS))
        mvpool = ctx.enter_context(tc.tile_pool(name="mv", bufs=TOK_BLOCKS))

        # constants: allocated up front; DMAs emitted after the first expert's
        # weight loads start so they don't steal startup HBM bandwidth.
        gam_t = const.tile([128, EMBED], F32)
        bet_t = const.tile([128, EMBED], F32)
        eps_t = const.tile([128, 1], F32)
        nc.vector.memset(eps_t, LN_EPS)
        idx_t = const.tile([128, TOK_BLOCKS, 2], I32)
        wesc_t = const.tile([128, NUM_EXPERTS * BLK], F32)
        var_all = const.tile([128, TOK_BLOCKS], F32)  # per-block variance
        rs_all = const.tile([128, TOK_BLOCKS], F32)
        warm_t = const.tile([128, 1], F32)
        nc.scalar.activation(out=warm_t, in_=eps_t, func=mybir.ActivationFunctionType.Silu, bias=0.0, scale=1.0)
        nc.scalar.activation(out=warm_t, in_=eps_t, func=mybir.ActivationFunctionType.Identity, bias=0.0, scale=1.0)

        s01_tiles: dict = {}

        def emit_gather_adds(b):
            """Gather the token block's two expert rows and sum with the
            residual. Emitted inside expert bounds[b]'s Wo loop, right after
            the ydram block write the gather actually depends on (prefs[b]),
            so the chain completes by the end of the expert's section."""
            xr_t = c2pool.tile([128, EMBED], F32, tag="xr")
            nc.sync.dma_start(out=xr_t, in_=xres[b])
            yrows = bounds[b] * BLK * 128 + prefs[b] * 128
            y1 = c2pool.tile([128, EMBED], BF16, tag="y1")
            nc.gpsimd.indirect_dma_start(
                out=y1,
                out_offset=None,
                in_=ydram[:yrows],
                in_offset=bass.IndirectOffsetOnAxis(ap=idx_t[:, b, 0:1], axis=0),
            )
            y2 = c2pool.tile([128, EMBED], BF16, tag="y2")
            nc.gpsimd.indirect_dma_start(
                out=y2,
                out_offset=None,
                in_=ydram[:yrows],
                in_offset=bass.IndirectOffsetOnAxis(ap=idx_t[:, b, 1:2], axis=0),
            )
            s01 = s01pool.tile([128, EMBED], F32, tag="s01")
            # tail blocks (bound = last expert) sum on vector so the adds run
            # in parallel with the gpsimd gathers after phase 1 ends
            eng_add = nc.vector if bounds[b] == NUM_EXPERTS - 1 else nc.gpsimd
            eng_add.tensor_tensor(out=s01, in0=y1, in1=y2, op=mybir.AluOpType.add)
            eng_add.tensor_tensor(out=s01, in0=s01, in1=xr_t, op=mybir.AluOpType.add)
            s01_tiles[b] = s01

        def emit_stats(b):
            """LN stats on vector, emitted at the end of expert bounds[b]'s
            section; by then the gather+add chain is already done, so this
            never head-of-line blocks the next expert's h-muls."""
            s01 = s01_tiles[b]
            stats = c2pool.tile([128, 6], F32, tag="st")
            nc.vector.bn_stats(out=stats, in_=s01)
            mv_b = mvpool.tile([128, 2], F32, tag="mv")
            nc.vector.bn_aggr(out=mv_b, in_=stats)
            nc.vector.tensor_scalar_mul(out=var_all[:, b : b + 1], in0=mv_b[:, 1:2], scalar1=1.0)
            s01_tiles[b] = (s01, mv_b)

        # Phase 1: per-expert SwiGLU MLP over the gathered token subsets.
        for e in range(NUM_EXPERTS):
            xt_t = xpool.tile([128, 4, C], F8, tag="xt")
            bvt_t = wpool.tile([128, 11], F32, tag="bvt")
            bgt8_t = wpool.tile([128, 11], F32, tag="bgt8")
            wv_t = wpool.tile([128, 4, HIDDEN_PAD], F8, tag="wv")
            wg_t = wpool.tile([128, 4, HIDDEN_PAD], F8, tag="wg")
            wo_t = wpool.tile([128, 11, EMBED], F8, tag="wo")
            if e == 0:
                # split the startup-critical loads so the first K-tiles'
                # matmuls can begin while the second halves stream in
                nc.sync.dma_start(out=xt_t[:, :2], in_=xt[e, :, :2])
                nc.sync.dma_start(out=wv_t[:, :2], in_=wv[e, :, :2])
                nc.sync.dma_start(out=xt_t[:, 2:], in_=xt[e, :, 2:])
                nc.sync.dma_start(out=wv_t[:, 2:], in_=wv[e, :, 2:])
                nc.sync.dma_start(out=bvt_t, in_=bvt[e])
                nc.sync.dma_start(out=bgt8_t, in_=bgt8[e])
                nc.sync.dma_start(out=wg_t, in_=wg[e])
                nc.sync.dma_start(out=wo_t, in_=wo[e])
            else:
                nc.sync.dma_start(out=xt_t, in_=xt[e])
                nc.sync.dma_start(out=bvt_t, in_=bvt[e])
                nc.sync.dma_start(out=bgt8_t, in_=bgt8[e])
                nc.sync.dma_start(out=wv_t, in_=wv[e])
                nc.sync.dma_start(out=wg_t, in_=wg[e])
                nc.sync.dma_start(out=wo_t, in_=wo[e])
            if e == 0:
                nc.scalar.dma_start(out=idx_t, in_=idx[:, :, :])
                nc.scalar.dma_start(out=wesc_t, in_=wesc[:, :])
                if ln_affine:
                    nc.scalar.dma_start(out=gam_t, in_=gam[:, :])
                    nc.scalar.dma_start(out=bet_t, in_=bet[:, :])

            # one h tile per Wo k-pair so each Wo matmul depends only on its
            # own pair's muls (a single h tile makes Wo j=0 falsely wait for
            # the m=10 elementwise chain, idling the PE ~6us per expert)
            h_pairs = [
                hpool.tile([128, 2, C], F8, tag=f"h{j}", name=f"h_pair{j}_{e}")
                for j in range(5)
            ]
            h10 = hpool.tile([128, C], F8, tag="h10")
            for m in range(11):
                psv = pvg.tile([128, C], F32, tag="psv")
                psg = pvg.tile([128, C], F32, tag="psg")
                for i in range(2):
                    nc.tensor.matmul(
                        psv,
                        lhsT=wv_t[:, 2 * i : 2 * i + 2, m * 128 : (m + 1) * 128],
                        rhs=xt_t[:, 2 * i : 2 * i + 2, :],
                        start=(i == 0),
                        stop=(i == 1),
                        perf_mode=DR,
                    )
                for i in range(2):
                    nc.tensor.matmul(
                        psg,
                        lhsT=wg_t[:, 2 * i : 2 * i + 2, m * 128 : (m + 1) * 128],
                        rhs=xt_t[:, 2 * i : 2 * i + 2, :],
                        start=(i == 0),
                        stop=(i == 1),
                        perf_mode=DR,
                    )
                v_t = vgpool.tile([128, C], BF16, tag="v")
                nc.scalar.activation(
                    out=v_t,
                    in_=psv,
                    func=mybir.ActivationFunctionType.Silu,
                    bias=bvt_t[:, m : m + 1],
                    scale=DEQ,
                )
                g_t = vgpool.tile([128, C], BF16, tag="g")
                if m % 2 == 1:
                    # scalar engine: Identity(psg/256 + 8*bg) == the g dequant
                    nc.scalar.activation(
                        out=g_t,
                        in_=psg,
                        func=mybir.ActivationFunctionType.Identity,
                        bias=bgt8_t[:, m : m + 1],
                        scale=SX * DEQ,
                    )
                else:
                    nc.vector.tensor_scalar(
                        out=g_t,
                        in0=psg,
                        scalar1=SX * DEQ,  # 1/256: folds the x8 used for h's fp8 range
                        scalar2=bgt8_t[:, m : m + 1],
                        op0=mybir.AluOpType.mult,
                        op1=mybir.AluOpType.add,
                    )
                h_dst = h10 if m == 10 else h_pairs[m // 2][:, m % 2, :]
                nc.vector.tensor_tensor(
                    out=h_dst, in0=v_t, in1=g_t, op=mybir.AluOpType.mult
                )

            y_all = ypool.tile([128, BLK, EMBED], BF16, tag="y")
            if C < BLK * 128:  # ragged last block: zero its column first
                nc.vector.memset(y_all[:, BLK - 1, :], 0.0)
            for blk in range(BLK):
                mb = min(128, C - blk * 128)  # last block is ragged
                pso = pop.tile([128, EMBED], F32, tag="pso")
                for j in range(5):
                    nc.tensor.matmul(
                        pso[:mb],
                        lhsT=h_pairs[j][:, :, blk * 128 : blk * 128 + mb],
                        rhs=wo_t[:, 2 * j : 2 * j + 2, :],
                        start=(j == 0),
                        stop=False,
                        perf_mode=DR,
                    )
                nc.tensor.matmul(
                    pso[:mb],
                    lhsT=h10[:, blk * 128 : blk * 128 + mb],
                    rhs=wo_t[:, 10, :],
                    start=False,
                    stop=True,
                )
                # PSUM evac with dequant*combine-weight scale; engine varies
                # per block to balance load (single scalar-AP ops are fast on
                # any engine)
                col = e * BLK + blk
                if blk == 0:
                    nc.scalar.activation(
                        out=y_all[:mb, blk, :],
                        in_=pso[:mb],
                        func=mybir.ActivationFunctionType.Identity,
                        bias=0.0,
                        scale=wesc_t[:mb, col : col + 1],
                    )
                else:
                    # gpsimd cannot read PSUM; vector takes blocks 1 and 2
                    nc.vector.tensor_scalar_mul(
                        out=y_all[:mb, blk, :],
                        in0=pso[:mb],
                        scalar1=wesc_t[:mb, col : col + 1],
                    )
                r0 = e * BLK * 128 + blk * 128
                nc.gpsimd.dma_start(out=ydram[r0 : r0 + 128, :], in_=y_all[:, blk, :])
                for b in range(TOK_BLOCKS):
                    if bounds[b] == e and prefs[b] == blk + 1:
                        emit_gather_adds(b)

        # Phase 2b: all LN stats deferred here so they never head-of-line
        # block an expert's h-muls on the vector queue; then batched rsqrt
        # (single Sqrt table load after all silus) and per-block normalize.
        for b in range(TOK_BLOCKS):
            emit_stats(b)
        nc.scalar.activation(
            out=rs_all,
            in_=var_all,
            func=mybir.ActivationFunctionType.Sqrt,
            bias=eps_t,
            scale=1.0,
        )
        nc.vector.reciprocal(out=rs_all, in_=rs_all)
        for b in range(TOK_BLOCKS):
            s01, mv_b = s01_tiles[b]
            # offset-0 [128,1] scalar APs only -- offset/strided scalar APs
            # trigger a ~4-8x DVE slow path on hardware
            rs_b = c2pool.tile([128, 1], F32, tag="rsb")
            nc.vector.tensor_scalar_mul(out=rs_b, in0=rs_all[:, b : b + 1], scalar1=1.0)
            nrm = c2pool.tile([128, EMBED], F32, tag=f"nrm{b % 2}")
            nc.vector.tensor_scalar(
                out=nrm,
                in0=s01,
                scalar1=mv_b[:, 0:1],
                scalar2=rs_b,
                op0=mybir.AluOpType.subtract,
                op1=mybir.AluOpType.mult,
            )
            if ln_affine:
                nc.vector.tensor_mul(out=nrm, in0=nrm, in1=gam_t)
                nc.vector.tensor_add(out=nrm, in0=nrm, in1=bet_t)
            nc.sync.dma_start(out=out[b], in_=nrm)

    nc.finalize()
    _NC_CACHE[key] = nc
    return nc


def prepare(x, router_w, Wv, bv, Wg, bg, Wo, bo, gamma, beta):
    """Host-side routing, balancing, quantization, per-core input build."""
    x = np.asarray(x)
    router_w = np.asarray(router_w, dtype=np.float32)
    Wv = np.asarray(Wv, dtype=np.float32)
    bv = np.asarray(bv, dtype=np.float32)
    Wg = np.asarray(Wg, dtype=np.float32)
    bg = np.asarray(bg, dtype=np.float32)
    Wo = np.asarray(Wo, dtype=np.float32)
    bo = np.asarray(bo, dtype=np.float32)
    gamma = np.asarray(gamma, dtype=np.float32)
    beta = np.asarray(beta, dtype=np.float32)

    orig_shape = x.shape
    flat = x.reshape(-1, EMBED).astype(np.float32)
    n = flat.shape[0]
    assert n == NCORE * TOK_PER_CORE

    e1, e2, w1, w2, assign, maxcnt = _route_and_assign(flat, router_w)
    C = max(256, ((maxcnt + 15) // 16) * 16)
    BLK = (C + 127) // 128

    # process experts in descending-load order: the last-processed expert is
    # the lightest, so fewer token blocks are bound to it and the combine tail
    # after the final expert shrinks
    loads = np.bincount(np.concatenate([e1, e2]), minlength=NUM_EXPERTS)
    perm = np.argsort(-loads, kind="stable")  # slot s processes expert perm[s]
    pos = np.empty(NUM_EXPERTS, np.int64)
    pos[perm] = np.arange(NUM_EXPERTS)

    # replicated weights, pre-tiled to [e, 128, ktiles, free]; fp8 with scale SW
    wv_r = np.zeros((NUM_EXPERTS, EMBED, HIDDEN_PAD), np.float32)
    wv_r[:, :, :HIDDEN_RAW] = Wv * SW
    wg_r = np.zeros((NUM_EXPERTS, EMBED, HIDDEN_PAD), np.float32)
    wg_r[:, :, :HIDDEN_RAW] = Wg * SW
    wo_r = np.zeros((NUM_EXPERTS, HIDDEN_PAD, EMBED), np.float32)
    wo_r[:, :HIDDEN_RAW, :] = Wo * SW
    wv_tiled = np.ascontiguousarray(
        wv_r.reshape(NUM_EXPERTS, 4, 128, HIDDEN_PAD).transpose(0, 2, 1, 3).astype(NP_F8)
    )
    wg_tiled = np.ascontiguousarray(
        wg_r.reshape(NUM_EXPERTS, 4, 128, HIDDEN_PAD).transpose(0, 2, 1, 3).astype(NP_F8)
    )
    wo_tiled = np.ascontiguousarray(
        wo_r.reshape(NUM_EXPERTS, 11, 128, EMBED).transpose(0, 2, 1, 3).astype(NP_F8)
    )
    bv_pad = np.zeros((NUM_EXPERTS, HIDDEN_PAD), np.float32)
    bv_pad[:, :HIDDEN_RAW] = bv
    bg_pad = np.zeros((NUM_EXPERTS, HIDDEN_PAD), np.float32)
    bg_pad[:, :HIDDEN_RAW] = bg * SX  # g is kept scaled by SX so h=v*g lands at 8*h
    # [e, 128, 11]: column m holds the bias slice for H-tile m on partitions
    bvt = np.ascontiguousarray(bv_pad.reshape(NUM_EXPERTS, 11, 128).transpose(0, 2, 1))
    bgt8 = np.ascontiguousarray(bg_pad.reshape(NUM_EXPERTS, 11, 128).transpose(0, 2, 1))
    # reindex weights/biases by processing slot
    wv_tiled = np.ascontiguousarray(wv_tiled[perm])
    wg_tiled = np.ascontiguousarray(wg_tiled[perm])
    wo_tiled = np.ascontiguousarray(wo_tiled[perm])
    bvt = np.ascontiguousarray(bvt[perm])
    bgt8 = np.ascontiguousarray(bgt8[perm])
    gam_rep = np.ascontiguousarray(np.broadcast_to(gamma, (128, EMBED)))
    bet_rep = np.ascontiguousarray(np.broadcast_to(beta, (128, EMBED)))

    # bo folded into the residual: xres = x + w1*bo[e1] + w2*bo[e2]
    bo_comb = w1[:, None] * bo[e1] + w2[:, None] * bo[e2]
    res_full = flat + bo_comb

    in_maps = []
    core_token_ids = []
    per_core_sorted = []
    block_bound = np.zeros(TOK_BLOCKS, np.int64)
    for c in range(NCORE):
        tok_c = np.nonzero(assign == c)[0]
        assert tok_c.size == TOK_PER_CORE
        # order the core's tokens by the latest processing SLOT they touch, so
        # early blocks' combines only depend on a prefix of the expert loop
        emax = np.maximum(pos[e1[tok_c]], pos[e2[tok_c]])
        order = np.argsort(emax, kind="stable")
        tok_c = tok_c[order]
        per_core_sorted.append(tok_c)
        emax_sorted = emax[order]
        for b in range(TOK_BLOCKS):
            block_bound[b] = max(block_bound[b], emax_sorted[(b + 1) * 128 - 1])
    bounds = tuple(int(v) for v in block_bound)
    # prefs[b]: how many 128-row blocks of expert bounds[b]'s slot-sorted rows
    # block b's gather needs (max over cores, block-aligned)
    block_pref = np.zeros(TOK_BLOCKS, np.int64)
    for c in range(NCORE):
        tok_c = per_core_sorted[c]
        for b in range(TOK_BLOCKS):
            eb = perm[bounds[b]]
            sel = (e1[tok_c] == eb) | (e2[tok_c] == eb)
            ids = np.nonzero(sel)[0]
            pref = int(np.searchsorted(ids, (b + 1) * 128))
            block_pref[b] = max(block_pref[b], (pref + 127) // 128)
    prefs = tuple(int(v) for v in block_pref)

    x8 = (flat * SX).astype(NP_F8)
    for c in range(NCORE):
        tok_c = per_core_sorted[c]
        core_token_ids.append(tok_c)
        # per-expert gathered token subsets (fp8, pre-scaled)
        xt_c = np.zeros((NUM_EXPERTS, EMBED, C), NP_F8)
        ridx = np.zeros((TOK_PER_CORE, 2), np.int64)
        wesc_c = np.zeros((128, NUM_EXPERTS * BLK), np.float32)
        for s in range(NUM_EXPERTS):
            ex = perm[s]
            sel1 = e1[tok_c] == ex
            sel2 = e2[tok_c] == ex
            # slot-sorted rows: slot s's row r serves the r-th smallest token
            # slot touching it, so a token block's gather needs only a prefix
            ids = np.nonzero(sel1 | sel2)[0]
            cnt = ids.size
            assert cnt <= C, (cnt, C)
            xt_c[s, :, :cnt] = x8[tok_c[ids]].T
            rows = s * BLK * 128 + np.arange(cnt)
            first = sel1[ids]
            ridx[ids[first], 0] = rows[first]
            ridx[ids[~first], 1] = rows[~first]
            wvals = np.where(first, w1[tok_c[ids]], w2[tok_c[ids]])
            wecol = np.zeros(BLK * 128, np.float32)
            wecol[:cnt] = wvals * DEQ
            wesc_c[:, s * BLK : (s + 1) * BLK] = wecol.reshape(BLK, 128).T
        in_map = {
            "xt": np.ascontiguousarray(
                xt_c.reshape(NUM_EXPERTS, 4, 128, C).transpose(0, 2, 1, 3)
            ),
            "wv": wv_tiled,
            "wg": wg_tiled,
            "wo": wo_tiled,
            "bvt": bvt,
            "bgt8": bgt8,
            "wesc": wesc_c,
            "gamma": gam_rep,
            "beta": bet_rep,
            "xres": np.ascontiguousarray(
                res_full[tok_c].reshape(TOK_BLOCKS, 128, EMBED)
            ),
            # [128, TOK_BLOCKS, 2]: [p, b, s] = ydram row for token b*128+p slot s
            "idx": np.ascontiguousarray(
                ridx.reshape(TOK_BLOCKS, 128, 2).transpose(1, 0, 2).astype(np.int32)
            ),
        }
        in_maps.append(in_map)

    ln_affine = not (np.all(gamma == 1.0) and np.all(beta == 0.0))
    return in_maps, C, bounds, prefs, ln_affine, core_token_ids, orig_shape


def assemble(results, core_token_ids, orig_shape):
    """Scatter per-core outputs back to full token order."""
    n = NCORE * TOK_PER_CORE
    out_full = np.zeros((n, EMBED), np.float32)
    for c in range(NCORE):
        out_full[core_token_ids[c]] = results[c]["out"].reshape(TOK_PER_CORE, EMBED)
    return out_full.reshape(orig_shape)


def kernel(x, router_w, Wv, bv, Wg, bg, Wo, bo, gamma, beta):
    in_maps, C, bounds, prefs, ln_affine, core_token_ids, orig_shape = prepare(
        x, router_w, Wv, bv, Wg, bg, Wo, bo, gamma, beta
    )
    nc = _build_nc(C, bounds, prefs, ln_affine)
    res = run_bass_kernel_spmd(nc, in_maps, list(range(NCORE)))
    return assemble(res.results, core_token_ids, orig_shape)
